# revision 1
# baseline (speedup 1.0000x reference)
"""Trainium2 Bass kernel for BehaviorLemming (two fused stencil steps).

Sharding: data-parallel over batch. B=16 across 8 cores -> 2 batches/core.
Layout: H rows in partitions, (channel, W) in the free dim; input is
streamed per 4-channel group (1MB DMAs) so sets pipeline smoothly.
Per row-tile: DVE computes masks and the products P=a*w (Q=b*w on
GPSIMD); PE applies the +-1 row shifts as bit-exact fp32 matmuls with
shifted identity matrices, accumulating S_up@Q + S_dn@P in PSUM; the
"no move" case is patched with copy_predicated (uint8 m0 mask) after
ScalarE evacuates PSUM. Both steps run on-chip; the intermediate world
never touches HBM.

H tiling: 4 main sets of 124 output rows per batch (128 input rows incl.
2-row circular halo each side), plus ONE merged set handling the last 16
rows of BOTH batches (b0 at partitions 0..19, b1 at 32..51, block-
diagonal shift matrices).
"""

import numpy as np

_PQPOOL = [None]

import concourse.bacc as bacc
import concourse.mybir as mybir
import concourse.tile as tile
from concourse.bass_utils import run_bass_kernel_spmd

B, C, H, W = 16, 20, 512, 512
N_CORES = 8
B_PER_CORE = B // N_CORES
ELEM_ID = 3.0
F32 = mybir.dt.float32
U8 = mybir.dt.uint8
NCH = 4                 # channels per PSUM group (4 banks; bufs=2 -> 8)
NGRP = C // NCH
GP_Q = True             # Q products go to GPSIMD
MAIN_OUT = 124          # output rows per main set
MERGED_B1_OFF = 32      # partition offset of batch 1 rows in the merged set
MERGED_NP = 52
DMA_SHIFT_GROUPS = (1, 3)   # step-2 groups whose shifts ride DMA instead of PE


def _load_rows(nc, dst_tile, src_ap, row_start, n_rows, p0=0):
    """Load n_rows (mod H, split at wrap) of src [NCH,H,W] into dst
    partitions [p0, p0+n_rows), free dim = (c, w)."""
    s = row_start % H
    remaining = n_rows
    while remaining > 0:
        n = min(remaining, H - s)
        src = src_ap[:, s : s + n, :].rearrange("c h w -> h c w")
        nc.sync.dma_start(out=dst_tile[p0 : p0 + n, :].rearrange(
            "h (c w) -> h c w", c=NCH), in_=src)
        p0 += n
        s = (s + n) % H
        remaining -= n


def _build_masks(nc, pool, pmain, su, sd, world_t, np_, shift_w):
    """Masks for one step. world_t's free dim starts with ch0 (elem ids)
    then ch1 (density). Returns (a_f32, b_f32, m0_u8) SBUF tiles."""
    al = mybir.AluOpType
    e = world_t[0:np_, 0:W]
    d = world_t[0:np_, W : 2 * W]

    # dR = roll(d, shift_w) along the free (W) axis
    dR = pool.tile([np_, W], F32, tag="dR")
    if shift_w == 1:
        nc.scalar.copy(dR[:, 1:W], d[:, 0 : W - 1])
        nc.scalar.copy(dR[:, 0:1], d[:, W - 1 : W])
    else:
        nc.scalar.copy(dR[:, 0 : W - 1], d[:, 1:W])
        nc.scalar.copy(dR[:, W - 1 : W], d[:, 0:1])

    # mask shift matmuls share one pmain slot: dA | dAR | b
    mp = pmain.tile([np_, NCH * W], F32, tag="ps")
    dA = mp[:, 0:W]
    dAR = mp[:, W : 2 * W]
    bp = mp[:, 2 * W : 3 * W]
    nc.tensor.matmul(out=dA, lhsT=su, rhs=d, start=True, stop=True)
    nc.tensor.matmul(out=dAR, lhsT=su, rhs=dR[:], start=True, stop=True)

    c1 = pool.tile([np_, W], F32, tag="c1")
    c2 = pool.tile([np_, W], F32, tag="c2")
    c3 = pool.tile([np_, W], F32, tag="c3")
    nc.vector.tensor_tensor(out=c1[:], in0=dR[:], in1=d, op=al.is_ge)
    nc.vector.tensor_tensor(out=c2[:], in0=dA, in1=d, op=al.is_lt)
    nc.vector.tensor_tensor(out=c3[:], in0=dAR, in1=d, op=al.is_lt)
    e3c3 = pool.tile([np_, W], F32, tag="e3")
    nc.vector.scalar_tensor_tensor(out=e3c3[:], in0=e, scalar=ELEM_ID,
                                   in1=c3[:], op0=al.is_equal,
                                   op1=al.logical_and)
    c12 = pool.tile([np_, W], F32, tag="c12")
    nc.vector.tensor_tensor(out=c12[:], in0=c1[:], in1=c2[:],
                            op=al.logical_and)
    a = pool.tile([np_, W], F32, tag="a")
    nc.vector.tensor_tensor(out=a[:], in0=c12[:], in1=e3c3[:],
                            op=al.logical_and)

    # b[p] = a[p+1]; evacuate to SBUF so the psum slot frees quickly
    nc.tensor.matmul(out=bp, lhsT=sd, rhs=a[:], start=True, stop=True)
    b = pool.tile([np_, W], F32, tag="b")
    nc.scalar.copy(b[:], bp)

    # m0 = (a | b) == 0, as uint8 for copy_predicated
    r = pool.tile([np_, W], F32, tag="r")
    nc.vector.tensor_tensor(out=r[:], in0=a[:], in1=b[:], op=al.logical_or)
    m0 = pool.tile([np_, W], U8, tag="m0")
    nc.vector.tensor_scalar(out=m0[:], in0=r[:], scalar1=0.0, scalar2=None,
                            op0=al.is_equal)
    return a, b, m0


def _step_combine(nc, pool, pmain, su, sd, src_g, a, b, m0, np_, dst_g,
                  shift_via_dma=False, q_on_dve=False):
    """One stencil step for one NCH-channel group:
    dst = m0 ? src : (S_up@(b*src) + S_dn@(a*src)).

    shift_via_dma: apply the row shifts with SBUF->SBUF accumulating DMAs
    instead of PE matmuls (dst rows 0 / np_-1 end up garbage; only legal
    when those rows are never consumed, i.e. step-2 output tiles)."""
    al = mybir.AluOpType
    fd = NCH * W
    src_v = src_g.rearrange("p (c w) -> p c w", c=NCH)
    a_b = a[:].unsqueeze(1).broadcast_to([np_, NCH, W])
    b_b = b[:].unsqueeze(1).broadcast_to([np_, NCH, W])
    m0_b = m0[:].unsqueeze(1).broadcast_to([np_, NCH, W])

    P = _PQPOOL[0].tile([np_, fd], F32, tag="P")
    Q = _PQPOOL[0].tile([np_, fd], F32, tag="Q")
    nc.vector.tensor_tensor(out=P[:].rearrange("p (c w) -> p c w", c=NCH),
                            in0=src_v, in1=a_b, op=al.mult)
    qeng = nc.vector if (q_on_dve or not GP_Q) else nc.gpsimd
    qeng.tensor_tensor(out=Q[:].rearrange("p (c w) -> p c w", c=NCH),
                       in0=src_v, in1=b_b, op=al.mult)

    if shift_via_dma:
        # dst[p] = Q[p-1]; then dst[p] += P[p+1]
        nc.gpsimd.dma_start(out=dst_g.tensor[1:np_, 0:fd],
                            in_=Q[0 : np_ - 1, :])
        nc.gpsimd.dma_start(out=dst_g.tensor[0 : np_ - 1, 0:fd],
                            in_=P[1:np_, :], accum_op=al.add)
    else:
        ps = pmain.tile([np_, fd], F32, tag="ps")
        for c in range(NCH):
            nc.tensor.matmul(out=ps[:, c * W : (c + 1) * W], lhsT=su,
                             rhs=Q[:, c * W : (c + 1) * W],
                             start=True, stop=False)
        for c in range(NCH):
            nc.tensor.matmul(out=ps[:, c * W : (c + 1) * W], lhsT=sd,
                             rhs=P[:, c * W : (c + 1) * W],
                             start=False, stop=True)
        nc.scalar.copy(dst_g, ps[:])
    nc.vector.copy_predicated(dst_g.rearrange("p (c w) -> p c w", c=NCH),
                              m0_b, src_v)


def _new_set_state(nc, pools, sd):
    """Allocate w1, load group 0 and build step-1 masks for a set."""
    wpool, bigpool, opool, pool, pmain = pools
    sup, sdn, np_, load_group, _sg = sd
    g0 = wpool.tile([128, NCH * W], F32, tag="w0g")
    load_group(g0, 0)
    masks1 = _build_masks(nc, pool, pmain, sup, sdn, g0, np_, 1)
    w1 = bigpool.tile([128, C * W], F32, tag="w1")
    return {"sd": sd, "g0": g0, "masks1": masks1, "w1": w1, "masks2": None}


def _step1_group(nc, pools, st, g):
    """Emit step-1 for one channel group of a set."""
    wpool, bigpool, opool, pool, pmain = pools
    sup, sdn, np_, load_group, _sg = st["sd"]
    a1, b1, m01 = st["masks1"]
    if g == 0:
        t = st["g0"]
    else:
        t = wpool.tile([128, NCH * W], F32, tag="w0g")
        load_group(t, g)
    dst = st["w1"][0:np_, g * NCH * W : (g + 1) * NCH * W]
    _step_combine(nc, pool, pmain, sup, sdn, t[0:np_, :],
                  a1, b1, m01, np_, dst, q_on_dve=(g == 0))
    if g == 0:
        # step-2 masks only need w1 ch0/ch1: emit now so the mask DVE
        # chain overlaps other groups' PE work
        st["masks2"] = _build_masks(nc, pool, pmain, sup, sdn,
                                    st["w1"], np_, -1)


def _step2_group(nc, pools, st, g):
    """Emit step-2 + store for one channel group of a set."""
    wpool, bigpool, opool, pool, pmain = pools
    sup, sdn, np_, _lg, store_group = st["sd"]
    a2, b2, m02 = st["masks2"]
    src = st["w1"][0:np_, g * NCH * W : (g + 1) * NCH * W]
    og = opool.tile([128, NCH * W], F32, tag="og")
    _step_combine(nc, pool, pmain, sup, sdn, src,
                  a2, b2, m02, np_, og[0:np_, :],
                  shift_via_dma=(g in DMA_SHIFT_GROUPS),
                  q_on_dve=(g == 0))
    store_group(og, g)


def build_kernel():
    nc = bacc.Bacc("TRN2", target_bir_lowering=False, debug=False,
                   num_devices=N_CORES)
    wd = nc.dram_tensor("world", [B_PER_CORE, C, H, W], F32,
                        kind="ExternalInput").ap()
    su_d = nc.dram_tensor("s_up", [128, 128], F32, kind="ExternalInput").ap()
    sd_d = nc.dram_tensor("s_dn", [128, 128], F32, kind="ExternalInput").ap()
    sum_d = nc.dram_tensor("s_up_m", [MERGED_NP, MERGED_NP], F32,
                           kind="ExternalInput").ap()
    sdm_d = nc.dram_tensor("s_dn_m", [MERGED_NP, MERGED_NP], F32,
                           kind="ExternalInput").ap()
    od = nc.dram_tensor("out", [B_PER_CORE, C, H, W], F32,
                        kind="ExternalOutput").ap()

    with tile.TileContext(nc) as tc:
        with (
            tc.tile_pool(name="const", bufs=1) as cpool,
            tc.tile_pool(name="wpool", bufs=3) as wpool,
            tc.tile_pool(name="big", bufs=2) as bigpool,
            tc.tile_pool(name="opool", bufs=2) as opool,
            tc.tile_pool(name="small", bufs=2) as pool,
            tc.tile_pool(name="pq", bufs=3) as pqpool,
            tc.tile_pool(name="pmain", bufs=2, space="PSUM") as pmain,
        ):
            st_up = cpool.tile([128, 128], F32)
            st_dn = cpool.tile([128, 128], F32)
            st_up_m = cpool.tile([MERGED_NP, MERGED_NP], F32)
            st_dn_m = cpool.tile([MERGED_NP, MERGED_NP], F32)
            nc.sync.dma_start(out=st_up[:], in_=su_d)
            nc.sync.dma_start(out=st_dn[:], in_=sd_d)
            nc.sync.dma_start(out=st_up_m[:], in_=sum_d)
            nc.sync.dma_start(out=st_dn_m[:], in_=sdm_d)

            _PQPOOL[0] = pqpool
            pools = (wpool, bigpool, opool, pool, pmain)

            def make_main_set(bi, si):
                r_out = si * MAIN_OUT

                def load_group(t, g):
                    src = wd[bi, g * NCH : (g + 1) * NCH]
                    _load_rows(nc, t, src, r_out - 2, 128)

                def store_group(og, g):
                    dst = od[bi, g * NCH : (g + 1) * NCH,
                             r_out : r_out + MAIN_OUT, :]
                    nc.sync.dma_start(
                        out=dst.rearrange("c h w -> h c w"),
                        in_=og[2 : 2 + MAIN_OUT, :].rearrange(
                            "h (c w) -> h c w", c=NCH))

                return (st_up[:], st_dn[:], 128, load_group, store_group)

            def make_merged_set():
                r_out = 4 * MAIN_OUT      # 496
                n_out = H - r_out         # 16

                def load_group(t, g):
                    # zero first (aligned range) so gap partitions between
                    # the batch blocks can't feed NaN garbage into the PE
                    nc.gpsimd.memset(t[0:64, :], 0.0)
                    for bi, p0 in ((0, 0), (1, MERGED_B1_OFF)):
                        src = wd[bi, g * NCH : (g + 1) * NCH]
                        _load_rows(nc, t, src, r_out - 2, n_out + 4, p0=p0)

                def store_group(og, g):
                    for bi, p0 in ((0, 2), (1, MERGED_B1_OFF + 2)):
                        dst = od[bi, g * NCH : (g + 1) * NCH,
                                 r_out : r_out + n_out, :]
                        nc.sync.dma_start(
                            out=dst.rearrange("c h w -> h c w"),
                            in_=og[p0 : p0 + n_out, :].rearrange(
                                "h (c w) -> h c w", c=NCH))

                return (st_up_m[:], st_dn_m[:], MERGED_NP, load_group,
                        store_group)

            sets = [make_main_set(bi, si)
                    for bi in range(B_PER_CORE) for si in range(4)]
            sets.append(make_merged_set())

            # software-pipelined emission: the NEXT set's g0 load + step-1
            # masks are emitted before the CURRENT set's step-2 groups
            st = _new_set_state(nc, pools, sets[0])
            _step1_group(nc, pools, st, 0)
            for i in range(len(sets)):
                for g in range(1, NGRP):
                    _step1_group(nc, pools, st, g)
                st_next = (_new_set_state(nc, pools, sets[i + 1])
                           if i + 1 < len(sets) else None)
                for g in range(NGRP):
                    _step2_group(nc, pools, st, g)
                    if g == 2 and st_next is not None:
                        # inject the next set's first step-1 group so the
                        # PE stream stays dense through the step-2 tail
                        _step1_group(nc, pools, st_next, 0)
                st = st_next

    nc.compile()
    return nc


def _shift_mats():
    s_up = np.zeros((128, 128), np.float32)  # out[m] = in[m-1]
    s_dn = np.zeros((128, 128), np.float32)  # out[m] = in[m+1]
    for m in range(128):
        if m >= 1:
            s_up[m - 1, m] = 1.0
        if m <= 126:
            s_dn[m + 1, m] = 1.0
    s_up_m = np.zeros((MERGED_NP, MERGED_NP), np.float32)
    s_dn_m = np.zeros((MERGED_NP, MERGED_NP), np.float32)
    for base in (0, MERGED_B1_OFF):
        for m in range(20):
            if m >= 1:
                s_up_m[base + m - 1, base + m] = 1.0
            if m <= 18:
                s_dn_m[base + m + 1, base + m] = 1.0
    return s_up, s_dn, s_up_m, s_dn_m


_NC_CACHE = {}


def kernel(world, rand_movement=None, rand_interact=None, rand_element=None,
           **_ignored):
    world = np.ascontiguousarray(world, dtype=np.float32)
    assert world.shape == (B, C, H, W), world.shape
    if "nc" not in _NC_CACHE:
        _NC_CACHE["nc"] = build_kernel()
    nc = _NC_CACHE["nc"]
    s_up, s_dn, s_up_m, s_dn_m = _shift_mats()
    in_maps = []
    for core in range(N_CORES):
        shard = world[core * B_PER_CORE : (core + 1) * B_PER_CORE]
        in_maps.append({"world": np.ascontiguousarray(shard),
                        "s_up": s_up, "s_dn": s_dn,
                        "s_up_m": s_up_m, "s_dn_m": s_dn_m})
    res = run_bass_kernel_spmd(nc, in_maps, list(range(N_CORES)),
                               trace=_NC_CACHE.get("trace", False))
    _NC_CACHE["last_result"] = res
    out = np.concatenate([r["out"] for r in res.results], axis=0)
    return out.astype(np.float32)


if __name__ == "__main__":
    rng = np.random.default_rng(0)
    w = rng.standard_normal((B, C, H, W)).astype(np.float32)
    w[:, 0] = rng.integers(0, 10, (B, 1, H, W)).astype(np.float32)[:, 0]
    out = kernel(w)
    print("ran:", out.shape, out.dtype)



# revision 2
# speedup vs baseline: 1.4985x; 1.4985x over previous
"""Trainium2 Bass kernel v2 for BehaviorLemming (two fused stencil steps).

Sharding: data-parallel over batch, B=16 across 8 cores -> 2 batches/core.
Layout: H rows in partitions, (channel, w) free. Per 128-row set both steps
run on-chip.

Key ideas vs v1:
- bf16 for 19 of 20 channels (tolerance 2e-2); ch1 (density) stays fp32 so
  the mask comparisons of both steps are bit-exact vs the fp32 reference.
- masks a,b,m0 materialized as int16 0x0000/0xFFFF; the three per-cell
  products P=a*w, Q=b*w, R=m0*w become ONE bitwise_and op per group with
  broadcast APs ([p,3,nch,512]): exact select-by-mask on bf16 bits.
- combine = 3 bf16 matmuls per channel: S_dn@P + S_up@Q + I@R (PSUM fp32
  accum). bf16 matmul is 4x cheaper than fp32 on PE. Exact at no-move and
  single-move cells; both-fire cells get the same one-add rounding as ref.
- ch1 path: products via arithmetic mult of (0/-1 i16) masks with fp32 d,
  fixed up with negated fp32 shift matrices; no-move cells restored with
  copy_predicated -> bit-exact density for step-2 masks.
- compares use shifted views (no dR materialization).
"""

import numpy as np

import concourse.bacc as bacc
import concourse.mybir as mybir
import concourse.tile as tile
from concourse.bass_utils import run_bass_kernel_spmd

B, C, H, W = 16, 20, 512, 512
N_CORES = 8
B_PER_CORE = B // N_CORES
ELEM_ID = 3.0
F32 = mybir.dt.float32
BF16 = mybir.dt.bfloat16
I16 = mybir.dt.int16
al = mybir.AluOpType

MAIN_OUT = 124
MERGED_B1_OFF = 32
MERGED_NP = 52

# bf16 channel order in w0b/w1b: [0, 2, 3, ..., 19]; group boundaries:
BS = [0, 3, 7, 11, 15, 19]
NGRP = 5

# engine assignment (tunable): products / input conversions per group
PROD_ENG = ["vector"] * 5
R_ON_POOL = [False, False, True, True, True]
CONV_ENG = ["scalar", "scalar", "scalar", "scalar", "vector"]
EVAC1_ENG = ["scalar"] * 5
EVAC2_ENG = ["scalar"] * 5
CMP_ENG = "vector"     # mask compares c1/c2/c3/e3/c12
CH1_ENG = "vector"     # ch1 products q1/p1


def _load_rows(nc, dst_tile, src_ap, row_start, n_rows, p0=0):
    """Load n_rows (mod H, split at wrap) of src [nch,H,W] into dst
    partitions [p0, p0+n_rows), free dim = (c, w)."""
    nch = src_ap.shape[0]
    s = row_start % H
    remaining = n_rows
    while remaining > 0:
        n = min(remaining, H - s)
        src = src_ap[:, s : s + n, :].rearrange("c h w -> h c w")
        nc.sync.dma_start(out=dst_tile[p0 : p0 + n, :].rearrange(
            "h (c w) -> h c w", c=nch), in_=src)
        p0 += n
        s = (s + n) % H
        remaining -= n


def _ecopy(nc, eng, out, in_):
    if eng == "scalar":
        nc.scalar.copy(out, in_)
    elif eng == "vector":
        nc.vector.tensor_copy(out, in_)
    else:
        nc.gpsimd.tensor_copy(out, in_)


class SetCtx:
    pass


def build_kernel():
    nc = bacc.Bacc("TRN2", target_bir_lowering=False, debug=False,
                   num_devices=N_CORES)
    wd = nc.dram_tensor("world", [B_PER_CORE, C, H, W], F32,
                        kind="ExternalInput").ap()
    od = nc.dram_tensor("out", [B_PER_CORE, C, H, W], F32,
                        kind="ExternalOutput").ap()

    mats = {}
    for name, shape, dt in [
        ("su_f", [128, 128], F32), ("sd_f", [128, 128], F32),
        ("su_b", [128, 128], BF16), ("sd_b", [128, 128], BF16),
        ("id_b", [128, 128], BF16),
        ("su_fm", [MERGED_NP, MERGED_NP], F32),
        ("sd_fm", [MERGED_NP, MERGED_NP], F32),
        ("su_bm", [MERGED_NP, MERGED_NP], BF16),
        ("sd_bm", [MERGED_NP, MERGED_NP], BF16),
        ("id_bm", [MERGED_NP, MERGED_NP], BF16),
    ]:
        mats[name] = nc.dram_tensor(name, shape, dt, kind="ExternalInput").ap()

    with tile.TileContext(nc) as tc:
        with (
            tc.tile_pool(name="const", bufs=1) as cpool,
            tc.tile_pool(name="lpool", bufs=2) as lpool,
            tc.tile_pool(name="bpool", bufs=2) as bpool,
            tc.tile_pool(name="w1pool", bufs=2) as w1pool,
            tc.tile_pool(name="mpool", bufs=2) as mpool,
            tc.tile_pool(name="pqpool", bufs=3) as pqpool,
            tc.tile_pool(name="opool", bufs=2) as opool,
            tc.tile_pool(name="pmain", bufs=4, space="PSUM") as pmain,
        ):
            cm = {}
            for name, shape, dt in [
                ("su_f", [128, 128], F32), ("sd_f", [128, 128], F32),
                ("su_b", [128, 128], BF16), ("sd_b", [128, 128], BF16),
                ("id_b", [128, 128], BF16),
                ("su_fm", [MERGED_NP, MERGED_NP], F32),
                ("sd_fm", [MERGED_NP, MERGED_NP], F32),
                ("su_bm", [MERGED_NP, MERGED_NP], BF16),
                ("sd_bm", [MERGED_NP, MERGED_NP], BF16),
                ("id_bm", [MERGED_NP, MERGED_NP], BF16),
            ]:
                t = cpool.tile(shape, dt, tag=name)
                nc.sync.dma_start(out=t[:], in_=mats[name])
                cm[name] = t

            def new_state(sd, do_convs=True):
                """Emit loads (+optionally conversions) for a set."""
                st = SetCtx()
                st.mk = {}
                st.pq = {}
                st.sd = sd
                st.np = sd["np"]
                st.merged = sd["merged"]
                sfx = "m" if st.merged else ""
                st.su_f = cm["su_f" + sfx][:]
                st.sd_f = cm["sd_f" + sfx][:]
                st.su_b = cm["su_b" + sfx][:]
                st.sd_b = cm["sd_b" + sfx][:]
                st.id_b = cm["id_b" + sfx][:]
                st.w0b = bpool.tile([128, 19 * W], BF16, tag="w0b")
                st.d0 = bpool.tile([128, W], F32, tag="d0")
                st.ltiles = []
                for g in range(NGRP):
                    t = lpool.tile([128, 4 * W], F32, tag="w0g")
                    sd["load"](t, g)
                    st.ltiles.append(t)
                if do_convs:
                    for g in range(NGRP):
                        emit_conv(st, g)
                return st

            def emit_conv(st, g):
                np_ = st.np
                t = st.ltiles[g]
                eng = CONV_ENG[g]
                if g == 0:
                    # ch0 -> w0b col0; ch2,3 -> w0b cols 1:3; ch1 -> d0 (f32)
                    _ecopy(nc, eng, st.w0b[0:np_, 0:W], t[0:np_, 0:W])
                    _ecopy(nc, eng, st.w0b[0:np_, W : 3 * W],
                           t[0:np_, 2 * W : 4 * W])
                    nc.scalar.copy(st.d0[0:np_, :], t[0:np_, W : 2 * W])
                else:
                    o = BS[g] * W
                    _ecopy(nc, eng, st.w0b[0:np_, o : o + 4 * W],
                           t[0:np_, 0 : 4 * W])
                st.ltiles[g] = None  # release reference

            def masks_pre(st, step):
                """Alloc mask tiles + dA shift matmul + evac to SBUF."""
                np_ = st.np
                mk = {"step": step, "sw": 1 if step == 1 else -1}
                if step == 1:
                    mk["e"] = st.w0b[0:np_, 0:W]
                    mk["d"] = st.d0[0:np_, :]
                else:
                    mk["e"] = st.w1b[0:np_, 0:W]
                    mk["d"] = st.w1d[0:np_, :]
                psm = pmain.tile([128, 2 * W], F32, tag="ps")
                dA = psm[0:np_, 0:W]
                nc.tensor.matmul(out=dA, lhsT=st.su_f, rhs=mk["d"],
                                 start=True, stop=True)
                dAe_t = mpool.tile([128, W], F32, tag="dAe")
                nc.scalar.copy(dAe_t[0:np_, :], dA)
                mk["dAe"] = dAe_t[0:np_, :]
                mk["m3"] = mpool.tile([128, 3 * W], BF16, tag="m3", name="m3t")
                st.mk[step] = mk

            def masks_cmp1(st, step):
                np_ = st.np
                mk = st.mk[step]
                d, sw = mk["d"], mk["sw"]
                cmp_e = getattr(nc, CMP_ENG)
                c1 = mpool.tile([128, W], BF16, tag="c1")
                if sw == 1:
                    cmp_e.tensor_tensor(out=c1[0:np_, 1:W],
                                        in0=d[:, 0 : W - 1], in1=d[:, 1:W],
                                        op=al.is_ge)
                    cmp_e.tensor_tensor(out=c1[0:np_, 0:1],
                                        in0=d[:, W - 1 : W], in1=d[:, 0:1],
                                        op=al.is_ge)
                else:
                    cmp_e.tensor_tensor(out=c1[0:np_, 0 : W - 1],
                                        in0=d[:, 1:W], in1=d[:, 0 : W - 1],
                                        op=al.is_ge)
                    cmp_e.tensor_tensor(out=c1[0:np_, W - 1 : W],
                                        in0=d[:, 0:1], in1=d[:, W - 1 : W],
                                        op=al.is_ge)
                mk["c1"] = c1

            def masks_cmp2(st, step):
                np_ = st.np
                mk = st.mk[step]
                d, sw, dAe = mk["d"], mk["sw"], mk["dAe"]
                cmp_e = getattr(nc, CMP_ENG)
                c2 = mpool.tile([128, W], BF16, tag="c2")
                c3 = mpool.tile([128, W], BF16, tag="c3")
                cmp_e.tensor_tensor(out=c2[0:np_, :], in0=dAe, in1=d,
                                    op=al.is_lt)
                if sw == 1:
                    cmp_e.tensor_tensor(out=c3[0:np_, 1:W],
                                        in0=dAe[:, 0 : W - 1],
                                        in1=d[:, 1:W], op=al.is_lt)
                    cmp_e.tensor_tensor(out=c3[0:np_, 0:1],
                                        in0=dAe[:, W - 1 : W],
                                        in1=d[:, 0:1], op=al.is_lt)
                else:
                    cmp_e.tensor_tensor(out=c3[0:np_, 0 : W - 1],
                                        in0=dAe[:, 1:W],
                                        in1=d[:, 0 : W - 1], op=al.is_lt)
                    cmp_e.tensor_tensor(out=c3[0:np_, W - 1 : W],
                                        in0=dAe[:, 0:1],
                                        in1=d[:, W - 1 : W], op=al.is_lt)
                mk["c2"], mk["c3"] = c2, c3

            def masks_join(st, step):
                np_ = st.np
                mk = st.mk[step]
                m3 = mk["m3"]
                a_ = m3[0:np_, 0:W]
                e3 = mpool.tile([128, W], BF16, tag="e3")
                c12 = mpool.tile([128, W], BF16, tag="c12")
                nc.vector.scalar_tensor_tensor(out=e3[0:np_, :], in0=mk["e"],
                                               scalar=ELEM_ID,
                                               in1=mk["c3"][0:np_, :],
                                               op0=al.is_equal,
                                               op1=al.logical_and)
                nc.gpsimd.tensor_tensor(out=c12[0:np_, :],
                                        in0=mk["c1"][0:np_, :],
                                        in1=mk["c2"][0:np_, :],
                                        op=al.mult)
                nc.gpsimd.tensor_tensor(out=a_, in0=c12[0:np_, :],
                                        in1=e3[0:np_, :], op=al.mult)
                psb = pmain.tile([128, 2 * W], F32, tag="ps")
                bps = psb[0:np_, 0:W]
                nc.tensor.matmul(out=bps, lhsT=st.sd_b, rhs=a_,
                                 start=True, stop=True)
                mk["bps"] = bps

            def masks_fin(st, step):
                np_ = st.np
                mk = st.mk[step]
                m3 = mk["m3"]
                d = mk["d"]
                a_ = m3[0:np_, 0:W]
                b_ = m3[0:np_, W : 2 * W]
                m0 = m3[0:np_, 2 * W : 3 * W]
                nc.scalar.copy(b_, mk["bps"])        # b: bf16 0/1
                s_ = mpool.tile([128, W], BF16, tag="s_")
                nc.gpsimd.tensor_tensor(out=s_[0:np_, :], in0=a_, in1=b_,
                                        op=al.add)
                nc.vector.tensor_scalar(out=m0, in0=s_[0:np_, :],
                                        scalar1=0.0, scalar2=None,
                                        op0=al.is_equal)
                m0i_t = mpool.tile([128, W], I16, tag="m0i")
                nc.vector.tensor_scalar(out=m0i_t[0:np_, :],
                                        in0=s_[0:np_, :], scalar1=0.0,
                                        scalar2=None, op0=al.is_equal)
                q1 = mpool.tile([128, W], F32, tag="q1")
                p1 = mpool.tile([128, W], F32, tag="p1")
                ch1_e = getattr(nc, CH1_ENG)
                ch1_e.tensor_tensor(out=q1[0:np_, :], in0=b_, in1=d,
                                    op=al.mult)
                ch1_e.tensor_tensor(out=p1[0:np_, :], in0=a_, in1=d,
                                    op=al.mult)
                psc = pmain.tile([128, 2 * W], F32, tag="ps")
                ch1 = psc[0:np_, 0:W]
                nc.tensor.matmul(out=ch1, lhsT=st.su_f, rhs=q1[0:np_, :],
                                 start=True, stop=False)
                nc.tensor.matmul(out=ch1, lhsT=st.sd_f, rhs=p1[0:np_, :],
                                 start=False, stop=True)
                nc.vector.copy_predicated(ch1, m0i_t[0:np_, :], d)
                if step == 1:
                    st.w1b = w1pool.tile([128, 19 * W], BF16, tag="w1b")
                    st.w1d = w1pool.tile([128, W], F32, tag="w1d")
                    nc.scalar.copy(st.w1d[0:np_, :], ch1)
                else:
                    st.og0 = opool.tile([128, 4 * W], F32, tag="og0")
                    nc.scalar.copy(st.og0[0:np_, W : 2 * W], ch1)
                st.m3 = m3

            def masks_all(st, step):
                masks_pre(st, step)
                masks_cmp1(st, step)
                masks_cmp2(st, step)
                masks_join(st, step)
                masks_fin(st, step)

            def emit_prod(st, gi, step):
                np_ = st.np
                nch = BS[gi + 1] - BS[gi]
                src_t = st.w0b if step == 1 else st.w1b
                src = src_t[0:np_, BS[gi] * W : BS[gi + 1] * W]
                fd = nch * W
                pq = pqpool.tile([128, 12 * W], BF16, tag="pqr",
                                 name="pqt")
                mv = (st.mk[step]["m3"][0:np_, :]
                      .rearrange("p (t w) -> p t w", t=3)
                      .unsqueeze(2).broadcast_to([np_, 3, nch, W]))
                sv = (src.rearrange("p (c w) -> p c w", c=nch)
                      .unsqueeze(1).broadcast_to([np_, 3, nch, W]))
                if R_ON_POOL[gi]:
                    # R = m0*src on Pool (early, off critical path);
                    # P,Q on DVE
                    m3t = st.mk[step]["m3"]
                    rmv = (m3t[0:np_, 2 * W : 3 * W]
                           .rearrange("p (t w) -> p t w", t=1)
                           .unsqueeze(2).broadcast_to([np_, 1, nch, W]))
                    rsv = (src.rearrange("p (c w) -> p c w", c=nch)
                           .unsqueeze(1).broadcast_to([np_, 1, nch, W]))
                    rov = (pq[0:np_, 2 * fd : 3 * fd]
                           .rearrange("p (t c w) -> p t c w", t=1, c=nch))
                    nc.gpsimd.tensor_tensor(out=rov, in0=rmv, in1=rsv,
                                            op=al.mult)
                    pmv = (m3t[0:np_, 0 : 2 * W]
                           .rearrange("p (t w) -> p t w", t=2)
                           .unsqueeze(2).broadcast_to([np_, 2, nch, W]))
                    psv = (src.rearrange("p (c w) -> p c w", c=nch)
                           .unsqueeze(1).broadcast_to([np_, 2, nch, W]))
                    pov = (pq[0:np_, 0 : 2 * fd]
                           .rearrange("p (t c w) -> p t c w", t=2, c=nch))
                    nc.vector.tensor_tensor(out=pov, in0=pmv, in1=psv,
                                            op=al.mult)
                else:
                    ov = (pq[0:np_, 0 : 3 * fd]
                          .rearrange("p (t c w) -> p t c w", t=3, c=nch))
                    eng = getattr(nc, PROD_ENG[gi])
                    eng.tensor_tensor(out=ov, in0=mv, in1=sv, op=al.mult)
                st.pq[(gi, step)] = pq

            def emit_mm(st, gi, step):
                np_ = st.np
                nch = BS[gi + 1] - BS[gi]
                fd = nch * W
                pq = st.pq.pop((gi, step))
                psA = pmain.tile([128, 2 * W], F32, tag="ps")
                psB = pmain.tile([128, 2 * W], F32, tag="ps")
                slots = [(psA, 0), (psA, 1), (psB, 0), (psB, 1)][:nch]
                # weight-grouped: one LdWeights per matrix per group
                for mi, mat in enumerate([st.sd_b, st.su_b, st.id_b]):
                    for c in range(nch):
                        blk = pq[0:np_,
                                 (mi * nch + c) * W : (mi * nch + c + 1) * W]
                        pt, half = slots[c]
                        oc = pt[0:np_, half * W : (half + 1) * W]
                        nc.tensor.matmul(out=oc, lhsT=mat, rhs=blk,
                                         start=(mi == 0), stop=(mi == 2))
                if step == 1:
                    o = BS[gi] * W
                    nA = min(nch, 2)
                    _ecopy(nc, EVAC1_ENG[gi],
                           st.w1b[0:np_, o : o + nA * W], psA[0:np_, 0:nA * W])
                    if nch > 2:
                        nB = nch - 2
                        _ecopy(nc, EVAC1_ENG[gi],
                               st.w1b[0:np_, o + 2 * W : o + nch * W],
                               psB[0:np_, 0:nB * W])
                else:
                    if gi == 0:
                        og = st.og0
                        # psA = [ch0, ch2], psB = [ch3]; ch1 comes from masks
                        _ecopy(nc, EVAC2_ENG[gi], og[0:np_, 0:W],
                               psA[0:np_, 0:W])
                        _ecopy(nc, EVAC2_ENG[gi], og[0:np_, 2 * W : 3 * W],
                               psA[0:np_, W : 2 * W])
                        _ecopy(nc, EVAC2_ENG[gi], og[0:np_, 3 * W : 4 * W],
                               psB[0:np_, 0:W])
                    else:
                        og = opool.tile([128, 4 * W], F32, tag="og")
                        _ecopy(nc, EVAC2_ENG[gi], og[0:np_, 0 : 2 * W],
                               psA[0:np_, :])
                        _ecopy(nc, EVAC2_ENG[gi], og[0:np_, 2 * W : 4 * W],
                               psB[0:np_, :])
                    st.sd["store"](og, gi)

            def make_main_set(bi, si):
                r_out = si * MAIN_OUT

                def load(t, g):
                    _load_rows(nc, t, wd[bi, 4 * g : 4 * g + 4], r_out - 2,
                               128)

                def store(og, gi):
                    chs = [0, 1, 2, 3] if gi == 0 else None
                    c0 = 4 * gi
                    dst = od[bi, c0 : c0 + 4, r_out : r_out + MAIN_OUT, :]
                    nc.sync.dma_start(
                        out=dst.rearrange("c h w -> h c w"),
                        in_=og[2 : 2 + MAIN_OUT, :].rearrange(
                            "h (c w) -> h c w", c=4))

                return {"np": 128, "merged": False, "load": load,
                        "store": store}

            def make_merged_set():
                r_out = 4 * MAIN_OUT      # 496
                n_out = H - r_out         # 16

                def load(t, g):
                    for bi, p0 in ((0, 0), (1, MERGED_B1_OFF)):
                        _load_rows(nc, t, wd[bi, 4 * g : 4 * g + 4],
                                   r_out - 2, n_out + 4, p0=p0)

                def store(og, gi):
                    c0 = 4 * gi
                    for bi, p0 in ((0, 2), (1, MERGED_B1_OFF + 2)):
                        dst = od[bi, c0 : c0 + 4, r_out : r_out + n_out, :]
                        nc.sync.dma_start(
                            out=dst.rearrange("c h w -> h c w"),
                            in_=og[p0 : p0 + n_out, :].rearrange(
                                "h (c w) -> h c w", c=4))

                return {"np": MERGED_NP, "merged": True, "load": load,
                        "store": store}

            sets = [make_main_set(bi, si)
                    for bi in range(B_PER_CORE) for si in range(4)]
            sets.append(make_merged_set())

            st = new_state(sets[0])
            masks_all(st, 1)
            for i in range(len(sets)):
                sn = (new_state(sets[i + 1], do_convs=False)
                      if i + 1 < len(sets) else None)
                # step-1: products ahead, dense PE trains, chain last
                emit_prod(st, 0, 1)
                emit_prod(st, 1, 1)
                emit_prod(st, 2, 1)
                masks_pre(st, 2)
                emit_mm(st, 0, 1)
                masks_cmp1(st, 2)
                emit_prod(st, 3, 1)
                emit_mm(st, 1, 1)
                masks_cmp2(st, 2)
                emit_prod(st, 4, 1)
                emit_mm(st, 2, 1)
                emit_mm(st, 3, 1)
                emit_mm(st, 4, 1)
                masks_join(st, 2)
                masks_fin(st, 2)
                # step-2: same, interleaving next set's conv + masks-1
                emit_prod(st, 0, 2)
                emit_prod(st, 1, 2)
                if sn is not None:
                    emit_conv(sn, 0)
                    emit_prod(st, 2, 2)
                    masks_pre(sn, 1)
                    emit_mm(st, 0, 2)
                    masks_cmp1(sn, 1)
                    emit_prod(st, 3, 2)
                    emit_conv(sn, 1)
                    emit_mm(st, 1, 2)
                    masks_cmp2(sn, 1)
                    emit_prod(st, 4, 2)
                    emit_conv(sn, 2)
                    emit_mm(st, 2, 2)
                    emit_conv(sn, 3)
                    emit_mm(st, 3, 2)
                    emit_conv(sn, 4)
                    emit_mm(st, 4, 2)
                    masks_join(sn, 1)
                    masks_fin(sn, 1)
                else:
                    emit_prod(st, 2, 2)
                    emit_mm(st, 0, 2)
                    emit_prod(st, 3, 2)
                    emit_mm(st, 1, 2)
                    emit_prod(st, 4, 2)
                    for g in range(2, NGRP):
                        emit_mm(st, g, 2)
                st = sn

    nc.compile()
    return nc


def _shift_mats():
    def mk(n, blocks):
        s_up = np.zeros((n, n), np.float32)   # out[m] = in[m-1]
        s_dn = np.zeros((n, n), np.float32)   # out[m] = in[m+1]
        for base, ln in blocks:
            for m in range(ln):
                if m >= 1:
                    s_up[base + m - 1, base + m] = 1.0
                if m <= ln - 2:
                    s_dn[base + m + 1, base + m] = 1.0
        return s_up, s_dn

    su, sd = mk(128, [(0, 128)])
    sum_, sdm = mk(MERGED_NP, [(0, 20), (MERGED_B1_OFF, 20)])
    ident = np.eye(128, dtype=np.float32)
    ident_m = np.eye(MERGED_NP, dtype=np.float32)

    def bf(x):
        import jax.numpy as jnp
        return np.asarray(jnp.asarray(x, dtype=jnp.bfloat16))

    return {
        "su_f": su, "sd_f": sd,
        "su_b": bf(su), "sd_b": bf(sd), "id_b": bf(ident),
        "su_fm": sum_, "sd_fm": sdm,
        "su_bm": bf(sum_), "sd_bm": bf(sdm), "id_bm": bf(ident_m),
    }


_NC_CACHE = {}


def kernel(world, rand_movement=None, rand_interact=None, rand_element=None,
           **_ignored):
    world = np.ascontiguousarray(world, dtype=np.float32)
    assert world.shape == (B, C, H, W), world.shape
    if "nc" not in _NC_CACHE:
        _NC_CACHE["nc"] = build_kernel()
    nc = _NC_CACHE["nc"]
    mats = _shift_mats()
    in_maps = []
    for core in range(N_CORES):
        shard = world[core * B_PER_CORE : (core + 1) * B_PER_CORE]
        m = {"world": np.ascontiguousarray(shard)}
        m.update(mats)
        in_maps.append(m)
    res = run_bass_kernel_spmd(nc, in_maps, list(range(N_CORES)),
                               trace=_NC_CACHE.get("trace", False))
    _NC_CACHE["last_result"] = res
    out = np.concatenate([r["out"] for r in res.results], axis=0)
    return out.astype(np.float32)


if __name__ == "__main__":
    rng = np.random.default_rng(0)
    w = rng.standard_normal((B, C, H, W)).astype(np.float32)
    w[:, 0] = rng.integers(0, 10, (B, 1, H, W)).astype(np.float32)[:, 0]
    out = kernel(w)
    print("ran:", out.shape, out.dtype)
